# revision 44
# baseline (speedup 1.0000x reference)
"""Trainium2 Bass kernel: packed-varlen causal GQA attention block.

Sharding: tensor-parallel across heads on 8 NeuronCores.
  core c: q-heads [4c, 4c+4), kv-head c.
  Phase 1: QKV projection (bf16 matmuls, fp32 accum) + RoPE -> qT/kT [d, tok], v [tok, d].
  Phase 2: flash-style attention in transposed layout: ST = K-tile^T stationary vs Q
           moving -> exp -> pT; attT = V-contract(pT); denominators via ones-matmul.
           Processed in q-chunks (<=512 tokens); PV/den matmuls skip fully-masked
           leading columns of diagonal key tiles.
  Phase 3: per-chunk AllGather of attT (bf16) across cores, wo one chunk behind
           so the collective overlaps attention/wo PE work.
Host only slices/casts/permutes inputs and concatenates the 8 output column slices.
"""

import sys

import numpy as np
import ml_dtypes

if "/opt/trn_rl_repo" not in sys.path:
    sys.path.insert(0, "/opt/trn_rl_repo")

BF16 = ml_dtypes.bfloat16

# Static problem config (matches the reference).
LENS = [1024, 896, 768, 512]
T = 3200
B = 4
DIM, NH, NKV, HD = 4096, 32, 8, 128
THETA = 500000.0
SCALE = 1.0 / float(np.sqrt(HD))
NCORES = 8
QH = NH // NCORES          # 4 q heads per core
QW = QH * HD               # 512 q/att feature cols per core
KC = DIM // 128            # 32 contraction chunks
SEQ_STARTS = [0, 1024, 1920, 2688]
NEG = -30000.0             # additive mask value; exp() underflows to 0

# Attention/AG/wo processing chunks: (seq, q0, width). Final chunks are 256
# wide so the non-overlappable tail (AG + wo of the last chunk) is small.
CHUNKS = [
    (0, 0, 512), (0, 512, 512),
    (1, 0, 512), (1, 512, 384),
    (2, 0, 512), (2, 512, 256),
    (3, 0, 256), (3, 256, 256),
]

_CACHE = {}


DEFAULT_VARIANT = ("fullint", "agseq", "awact")


def _build_program(phases=(1, 2, 3), collective=True, repeat=1,
                   variant=DEFAULT_VARIANT):
    import concourse.mybir as mybir
    import concourse.tile as tile
    from concourse import bacc

    f32 = mybir.dt.float32
    bf16 = mybir.dt.bfloat16

    nc = bacc.Bacc("TRN2", target_bir_lowering=False, debug=False,
                   enable_asserts=False, num_devices=NCORES)

    # ---- I/O ----
    xT_d = nc.dram_tensor("xT", [DIM, T], bf16, kind="ExternalInput")
    wq_d = nc.dram_tensor("wq", [128, KC, QW], bf16, kind="ExternalInput")
    wk_d = nc.dram_tensor("wk", [128, KC, HD], bf16, kind="ExternalInput")
    wv_d = nc.dram_tensor("wv", [128, KC, HD], bf16, kind="ExternalInput")
    wo_d = nc.dram_tensor("wo", [128, KC, QW], bf16, kind="ExternalInput")
    cs_d = nc.dram_tensor("cs", [128, T], f32, kind="ExternalInput")
    tri_d = nc.dram_tensor("tri", [128, 128], bf16, kind="ExternalInput")
    out_d = nc.dram_tensor("out", [T, QW], bf16, kind="ExternalOutput")

    CHUNK = 256  # phase-1 token chunk

    per_seq_ag = "agseq" in variant
    narrow = "nonarrow" not in variant
    pump_between_heads = "nopump" not in variant

    with tile.TileContext(nc) as tc:
        with (
            tc.tile_pool(name="sb", bufs=1) as sb,
            tc.tile_pool(name="ps", bufs=2, space="PSUM") as ps,
            tc.tile_pool(name="dram", bufs=1, space="DRAM") as dpool,
        ):
            # ---- resident SBUF tensors (allocated now, DMAs issued after the
            # first phase-1 chunk's loads so the PE can start ~20us sooner) ----
            wk_sb = sb.tile([128, KC, HD], bf16)
            wv_sb = sb.tile([128, KC, HD], bf16)
            cs_sb = sb.tile([128, T], f32)  # cos on parts 0-63, sin on 64-127
            tri_sb = sb.tile([128, 128], bf16)  # 0/1 causal keep-mask
            ones_sb = sb.tile([128, 128], bf16)
            nc.vector.memset(ones_sb[:], 1.0)
            id_sb = sb.tile([128, 128], bf16)
            from concourse.masks import make_identity
            make_identity(nc, id_sb[:])

            qT_sb = sb.tile([128, QH, T], bf16)   # per q-head [d, tok], roped+scaled
            kT_sb = sb.tile([128, T], bf16)       # kv head   [d, tok], roped
            v_sb = sb.tile([128, T], bf16)        # [tok-part, d] per 128-token tile

            xT_r = xT_d.ap().rearrange("(a p) t -> p a t", p=128)

            def rope(dst0, dst1, psum, t0, w):
                """dst0/dst1: [64, w] bf16 slices; psum [128, w] f32 (q or k chunk)."""
                p0 = psum[0:64, :]
                p1 = psum[64:128, :]
                cw = cs_sb[0:64, t0:t0 + w]
                sw = cs_sb[64:128, t0:t0 + w]
                m0 = sb.tile([64, CHUNK], f32, tag="rtmp", bufs=4)
                nc.vector.tensor_mul(m0[:, :w], p0, cw)
                m1 = sb.tile([64, CHUNK], f32, tag="rtmp", bufs=4)
                nc.vector.tensor_mul(m1[:, :w], p1, sw)
                nc.vector.tensor_sub(dst0, m0[:, :w], m1[:, :w])
                m2 = sb.tile([64, CHUNK], f32, tag="rtmp", bufs=4)
                nc.vector.tensor_mul(m2[:, :w], p0, sw)
                m3 = sb.tile([64, CHUNK], f32, tag="rtmp", bufs=4)
                nc.vector.tensor_mul(m3[:, :w], p1, cw)
                nc.vector.tensor_add(dst1, m2[:, :w], m3[:, :w])

            resident_loaded = [False]

            def load_residents():
                nc.sync.dma_start(wk_sb[:], wk_d.ap())
                nc.sync.dma_start(cs_sb[:], cs_d.ap())
                nc.sync.dma_start(wv_sb[:], wv_d.ap())
                nc.sync.dma_start(tri_sb[:], tri_d.ap())
                resident_loaded[0] = True

            for _rep in range(repeat):
                # ---- Phase 2+3 config (needed up front: fullint interleaves
                # attention chunks into the phase-1 stream) ----
                work = CHUNKS
                # AG granularity: one gather per chunk, or one per packed seq
                # (fewer, larger collectives — they execute strictly serially)
                agmix = "agmix" in variant
                if per_seq_ag or agmix:
                    nseq = B - 1 if agmix else B
                    ag_groups = [(b, 0, LENS[b],
                                  [ci for ci, (b2, _, _) in enumerate(work)
                                   if b2 == b]) for b in range(nseq)]
                    ag_groups += [(b, q0, w, [ci])
                                  for ci, (b, q0, w) in enumerate(work)
                                  if b >= nseq]
                else:
                    ag_groups = [(b, q0, w, [ci])
                                 for ci, (b, q0, w) in enumerate(work)]
                gi_of_chunk = {}      # chunk -> its group
                fire_after = {}       # last chunk of group -> group index
                for gi, (_, _, _, cis) in enumerate(ag_groups):
                    for ci in cis:
                        gi_of_chunk[ci] = gi
                    fire_after[cis[-1]] = gi
                ag_ins = [dpool.tile([QW, span], bf16, tag=f"agin{gi}",
                                     name=f"agin{gi}")
                          for gi, (b, gq0, span, _) in enumerate(ag_groups)] \
                    if 2 in phases or 3 in phases else []

                # wo pump state: units (one aw DMA covering <=AWTOK tokens)
                # -> groups (one 128-token PSUM accumulation, 32 matmuls).
                # min_slot gates a unit's aw DMA until its AllGather has had
                # enough pump points to complete (avoids head-of-line stalls
                # of the in-order DMA queue).
                fullint = "fullint" in variant and 1 in phases and 2 in phases
                AWTOK = 128 if fullint else 256
                unit_q = []    # (gi, t0, min_slot) not yet DMA-issued
                group_q = []   # (aw_tile, wl, s, out_base) ready to emit
                wo_sb_box = [None]
                slot_box = [0]
                ag_outs = {}
                pushed_gis = set()

                def emit_group(aw, wl, s, out_base):
                    op = ps.tile([128, 512], f32, tag="D", bufs=2, name="op")
                    for kc in range(KC):
                        nc.tensor.matmul(
                            op[:], aw[:, kc, s * 128:(s + 1) * 128],
                            wo_sb_box[0][:, kc, :],
                            start=(kc == 0), stop=(kc == KC - 1))
                    os_ = sb.tile([128, 512], bf16, tag="os", bufs=2,
                                  name="os_")
                    nc.any.tensor_copy(os_[:], op[:])
                    os_eng = nc.gpsimd if "aopool" in variant else nc.sync
                    os_eng.dma_start(
                        out_d.ap()[out_base + s * 128:out_base + (s + 1) * 128,
                                   :], os_[:])

                def pump_wo(n_groups, final=False):
                    slot_box[0] += 1
                    emitted = 0
                    while emitted < n_groups:
                        if group_q:
                            emit_group(*group_q.pop(0))
                            emitted += 1
                        elif unit_q and (final
                                         or unit_q[0][2] <= slot_box[0]):
                            gi_, t0_, _ = unit_q.pop(0)
                            b_, gq0_, span_, _ = ag_groups[gi_]
                            wl = min(AWTOK, span_ - t0_)
                            ag_r = ag_outs[gi_].rearrange("(a p) t -> p a t",
                                                          p=128)
                            aw = sb.tile([128, KC, AWTOK], bf16, tag="aw",
                                         bufs=2, name="aw")
                            aw_eng = nc.scalar if "awact" in variant else nc.sync
                            aw_eng.dma_start(aw[:, :, :wl],
                                             ag_r[:, :, t0_:t0_ + wl])
                            base = SEQ_STARTS[b_] + gq0_ + t0_
                            for s in range(wl // 128):
                                group_q.append((aw, wl, s, base))
                        else:
                            break

                def push_units(gi_, gate):
                    span_ = ag_groups[gi_][2]
                    unit_q.extend((gi_, t0, slot_box[0] + gate)
                                  for t0 in range(0, span_, AWTOK))

                def attn_chunk(ci):
                    b, q0, w = work[ci]
                    s0 = SEQ_STARTS[b]
                    gi = gi_of_chunk[ci]
                    ag_in = ag_ins[gi]
                    gq0 = ag_groups[gi][1]
                    nkt = (q0 + w) // 128
                    pump_ok = wo_sb_box[0] is not None and (fullint or ci >= 2)
                    for h in range(QH):
                        pts = []
                        offs = []
                        for kb in range(nkt):
                            k0 = kb * 128
                            pt = sb.tile([128, 512], bf16, tag="pT", bufs=12)
                            if k0 + 128 <= q0:
                                st = ps.tile([128, 512], f32, tag="A", bufs=3)
                                nc.tensor.matmul(
                                    st[:, :w], kT_sb[:, s0 + k0:s0 + k0 + 128],
                                    qT_sb[:, h, s0 + q0:s0 + q0 + w],
                                    start=True, stop=True)
                                nc.scalar.activation(
                                    pt[:, :w], st[:, :w],
                                    mybir.ActivationFunctionType.Exp)
                                offs.append(0)
                            else:
                                off = k0 - q0
                                wd = w - off
                                st = ps.tile([128, 512], f32, tag="A", bufs=3)
                                nc.tensor.matmul(
                                    st[:, :wd], kT_sb[:, s0 + k0:s0 + k0 + 128],
                                    qT_sb[:, h, s0 + k0:s0 + k0 + wd],
                                    start=True, stop=True)
                                wm = min(128, wd)
                                if not narrow and off > 0:
                                    nc.vector.memset(pt[:, :off], 0.0)
                                nc.scalar.activation(
                                    pt[:, off:off + wd], st[:, :wd],
                                    mybir.ActivationFunctionType.Exp)
                                # zero the invalid triangle post-exp:
                                # bf16 4x-mode mul, off the psum path
                                nc.vector.tensor_mul(
                                    pt[:, off:off + wm],
                                    pt[:, off:off + wm], tri_sb[:, :wm])
                                offs.append(off if narrow else 0)
                            pts.append(pt)

                        # one wo group here keeps the PE busy while the ACT
                        # engine computes exp over this head's score tiles
                        if 3 in phases and pump_ok and pump_between_heads:
                            pump_wo(2 if (per_seq_ag or agmix) else 1)

                        att = ps.tile([128, 512], f32, tag="B", bufs=2)
                        den = ps.tile([128, 512], f32, tag="C", bufs=1)
                        for j in range(nkt):
                            o = offs[j] if j > 0 else 0
                            fl = dict(start=(j == 0), stop=(j == nkt - 1),
                                      skip_group_check=True)
                            nc.tensor.matmul(
                                att[:, o:w],
                                v_sb[:, s0 + j * 128:s0 + (j + 1) * 128],
                                pts[j][:, o:w], **fl)
                            nc.tensor.matmul(den[:, o:w], ones_sb[:],
                                             pts[j][:, o:w], **fl)
                        rec = sb.tile([128, 512], f32, tag="rec", bufs=2)
                        # DVE exact reciprocal is an 8-deep iterative divide
                        # (~8x slower); 18-bit approx is plenty for a softmax
                        # denom feeding bf16.
                        nc.vector.reciprocal_approx_fast(rec[:, :w],
                                                         den[:, :w])
                        ao = sb.tile([128, 512], bf16, tag="ao", bufs=2)
                        nc.vector.tensor_mul(ao[:, :w], att[:, :w],
                                             rec[:, :w])
                        ao_eng = nc.gpsimd if "aopool" in variant else nc.sync
                        ao_eng.dma_start(
                            ag_in[h * HD:(h + 1) * HD,
                                  q0 - gq0:q0 - gq0 + w], ao[:, :w])

                    # ---- AllGather when this chunk completes its group; wo
                    # units enter the pump queue with >=1 chunk of runway so
                    # the collective completes under attention+wo PE work ----
                    if 3 in phases:
                        if ci == 0 and not fullint:
                            # overlaps attention; slot shared with wq (tag bigw)
                            wo_sb_box[0] = sb.tile([128, KC, QW], bf16,
                                                   tag="bigw", name="wo_sb")
                            nc.sync.dma_start(wo_sb_box[0][:], wo_d.ap())
                        fgi = fire_after.get(ci)
                        if fgi is not None:
                            b_, gq0_, span_, _ = ag_groups[fgi]
                            ag_out = dpool.tile(
                                [NH * HD, span_], bf16, tag=f"agout{fgi}",
                                name=f"agout{fgi}",
                                addr_space="Shared" if collective else "Local")
                            if collective:
                                nc.gpsimd.collective_compute(
                                    "AllGather",
                                    mybir.AluOpType.bypass,
                                    replica_groups=[list(range(NCORES))],
                                    ins=[ag_ins[fgi].opt()],
                                    outs=[ag_out.opt()],
                                )
                            else:
                                # collective-free stand-in: one local copy with
                                # the same per-core payload (timing ablation;
                                # other cores' slices are garbage unless repl8)
                                reps = NCORES if "repl8" in variant else 1
                                for r in range(reps):
                                    nc.sync.dma_start(
                                        ag_out[r * QW:(r + 1) * QW, :],
                                        ag_ins[fgi][:])
                            ag_outs[fgi] = ag_out
                            # chunked AGs get one extra group of runway before
                            # their readback DMA is issued; per-seq AGs are
                            # consumed from the next chunk on. fullint gates
                            # on pump slots instead (AG completes under the
                            # interleaved phase-1 stream).
                            pgi = (fgi if (per_seq_ag or fullint or agmix)
                                   else fgi - 1)
                            if pgi >= 0 and pgi not in pushed_gis:
                                pushed_gis.add(pgi)
                                if fullint:
                                    gate = 3
                                elif per_seq_ag or agmix:
                                    # per-seq AGs are ~1MB (14-20us): give
                                    # them ~1.5 chunks of pump slots before
                                    # the readback DMA may issue
                                    gate = 6
                                else:
                                    gate = 0
                                push_units(pgi, gate)
                                if not pump_between_heads:
                                    pump_wo(10 ** 9, final=True)

                # ---- Phase 1: QKV + RoPE (+ interleaved attention chunks
                # and wo groups in fullint mode, so the AllGathers complete
                # under phase-1's dense PE stream) ----
                if 1 in phases:
                    # wq/wo share one SBUF slot (tag bigw) unless fullint
                    # (interleaving needs both resident: wo gets tag wot).
                    wq_sb = sb.tile([128, KC, QW], bf16, tag="bigw")
                    if fullint and 3 in phases:
                        wo_sb_box[0] = sb.tile([128, KC, QW], bf16,
                                               tag="wot", name="wo_sb")
                ready_at = {}
                if fullint:
                    for ci2, (b2, q02, w2) in enumerate(work):
                        endtok = SEQ_STARTS[b2] + q02 + w2
                        k = -(-endtok // CHUNK) - 1
                        ready_at.setdefault(k, []).append(ci2)
                for ki, t0 in (enumerate(range(0, T, CHUNK))
                               if 1 in phases else []):
                    w = min(CHUNK, T - t0)
                    xt = sb.tile([128, KC, CHUNK], bf16, tag="xt", bufs=2)
                    if t0 == 0:
                        # interleave wq pieces with the chunk-0 xt pieces so
                        # matmul kc can start once pieces kc//8 have landed
                        for pc in range(0, KC, 8):
                            nc.sync.dma_start(wq_sb[:, pc:pc + 8, :],
                                              wq_d.ap()[:, pc:pc + 8, :])
                            nc.sync.dma_start(xt[:, pc:pc + 8, :w],
                                              xT_r[:, pc:pc + 8, t0:t0 + w])
                        if not resident_loaded[0]:
                            load_residents()
                    else:
                        for pc in range(0, KC, 8):
                            nc.sync.dma_start(xt[:, pc:pc + 8, :w],
                                              xT_r[:, pc:pc + 8, t0:t0 + w])
                    if fullint and 3 in phases and ki == 2:
                        for pc in range(0, KC, 8):
                            nc.sync.dma_start(wo_sb_box[0][:, pc:pc + 8, :],
                                              wo_d.ap()[:, pc:pc + 8, :])

                    for h in range(QH):
                        qp = ps.tile([128, 512], f32, tag="A", bufs=3)
                        for kc in range(KC):
                            nc.tensor.matmul(
                                qp[:, :w],
                                wq_sb[:, kc, h * HD:(h + 1) * HD],
                                xt[:, kc, :w],
                                start=(kc == 0), stop=(kc == KC - 1),
                            )
                        rope(qT_sb[0:64, h, t0:t0 + w],
                             qT_sb[64:128, h, t0:t0 + w], qp[:, :w], t0, w)

                    kp = ps.tile([128, 512], f32, tag="A", bufs=3)
                    for kc in range(KC):
                        nc.tensor.matmul(kp[:, :w], wk_sb[:, kc, :],
                                         xt[:, kc, :w],
                                         start=(kc == 0), stop=(kc == KC - 1))
                    rope(kT_sb[0:64, t0:t0 + w], kT_sb[64:128, t0:t0 + w],
                         kp[:, :w], t0, w)

                    # V: compute vT [d, tok] with N=w moving (fast), then
                    # PE-transpose each 128-token tile to [tok, d]. The bf16
                    # round-trip through the transpose is exact.
                    vp = ps.tile([128, 512], f32, tag="A", bufs=3, name="vp")
                    for kc in range(KC):
                        nc.tensor.matmul(vp[:, :w], wv_sb[:, kc, :],
                                         xt[:, kc, :w],
                                         start=(kc == 0), stop=(kc == KC - 1))
                    vt_sb = sb.tile([128, CHUNK], bf16, tag="vt", bufs=2)
                    nc.any.tensor_copy(vt_sb[:, :w], vp[:, :w])
                    for s in range(w // 128):
                        tp = ps.tile([128, 128], bf16, tag="B", bufs=2,
                                     name="tp")
                        nc.tensor.transpose(
                            tp[:], vt_sb[:, s * 128:(s + 1) * 128], id_sb[:])
                        nc.any.tensor_copy(
                            v_sb[:, t0 + s * 128:t0 + (s + 1) * 128], tp[:])

                    if fullint:
                        for ci2 in ready_at.get(ki, []):
                            attn_chunk(ci2)
                        if 3 in phases:
                            pump_wo(1)
                    # demoag: dummy fire-and-forget AllGathers spread through
                    # phase 1 — measures the unhideable serial cost per AG
                    if "demoag" in variant and 2 <= ki <= 9:
                        if ki == 2:
                            dag_src = dpool.tile([QW, 1024], bf16,
                                                 tag="dag_in", name="dag_in")
                            nc.sync.dma_start(dag_src[:],
                                              xT_d.ap()[0:QW, 0:1024])
                        dag_out = dpool.tile(
                            [NH * HD, 1024], bf16, tag=f"dag_out{ki}",
                            name=f"dag_out{ki}", addr_space="Shared")
                        nc.gpsimd.collective_compute(
                            "AllGather", mybir.AluOpType.bypass,
                            replica_groups=[list(range(NCORES))],
                            ins=[dag_src.opt()], outs=[dag_out.opt()])

                if 1 not in phases and not resident_loaded[0]:
                    load_residents()

                if 2 in phases and not fullint:
                    for ci2 in range(len(work)):
                        attn_chunk(ci2)

                if 3 in phases and 2 in phases:
                    for gi2 in range(len(ag_groups)):
                        if gi2 not in pushed_gis:
                            pushed_gis.add(gi2)
                            push_units(gi2, 0)
                    pump_wo(10 ** 9, final=True)

    nc.compile()
    return nc


def _host_prep(x, wq, wk, wv, wo, positions):
    """Per-core input maps: slice per head group, permute rope pairs, cast bf16."""
    # rope pair permutation within each head: evens then odds
    perm = np.concatenate([np.arange(0, HD, 2), np.arange(1, HD, 2)])

    inv_freq = 1.0 / (THETA ** (np.arange(64, dtype=np.float64) * 2.0 / HD))
    ang = positions.astype(np.float64)[None, :] * inv_freq[:, None]  # [64, T]
    cs_t = np.ascontiguousarray(np.concatenate(
        [np.cos(ang), np.sin(ang)]).astype(np.float32))  # [128, T]

    tri = np.where(np.arange(128)[None, :] >= np.arange(128)[:, None],
                   1.0, 0.0).astype(BF16)

    xT = np.ascontiguousarray(x.T.astype(BF16))

    def shard_w(w_full, cols, permute):
        ws = w_full[:, cols].astype(np.float64)
        if permute is not None:
            nh = ws.shape[1] // HD
            ws = ws.reshape(DIM, nh, HD)[:, :, permute].reshape(DIM, nh * HD)
        return ws

    in_maps = []
    for c in range(NCORES):
        qcols = slice(c * QW, (c + 1) * QW)
        kcols = slice(c * HD, (c + 1) * HD)
        wq_c = shard_w(wq, qcols, perm) * SCALE
        wk_c = shard_w(wk, kcols, perm)
        wv_c = wv[:, kcols].astype(np.float64)
        wo_c = wo[:, qcols].astype(np.float64)

        def lay(wm):  # [DIM, n] -> [128, KC, n] with dim = a*128+p
            n = wm.shape[1]
            return np.ascontiguousarray(
                wm.reshape(KC, 128, n).transpose(1, 0, 2).astype(BF16))

        in_maps.append({
            "xT": xT,
            "wq": lay(wq_c),
            "wk": lay(wk_c),
            "wv": lay(wv_c),
            "wo": lay(wo_c),
            "cs": cs_t,
            "tri": tri,
        })
    return in_maps


def _get_program():
    if "nc" not in _CACHE:
        _CACHE["nc"] = _build_program()
    return _CACHE["nc"]


def kernel(x, wq, wk, wv, wo, positions, _trace=False):
    from concourse import bass_utils

    nc = _get_program()
    in_maps = _host_prep(np.asarray(x), np.asarray(wq), np.asarray(wk),
                         np.asarray(wv), np.asarray(wo), np.asarray(positions))
    res = bass_utils.run_bass_kernel_spmd(
        nc, in_maps, core_ids=list(range(NCORES)), trace=_trace)
    _CACHE["last_result"] = res
    out = np.concatenate([res.results[c]["out"] for c in range(NCORES)], axis=1)
    return np.ascontiguousarray(out.astype(np.float32))
